# revision 67
# baseline (speedup 1.0000x reference)
"""Trainium2 Bass kernel for nn_ChamferDistanceL2.

Math notes (exact reformulation of the reference):
  probs = softmax(logits) over V; the chamfer "y" cloud is one-hot rows of
  targets (masked), so the pairwise squared distances collapse to
      d2[b,i,j] = xs_i + mask_j - 2*mask_i*mask_j*probs[b,i,t_j]
  with xs_i = mask_i * sum_{v>=1} probs[b,i,v]^2.  Everything the device
  needs from the full [B,S,V] logits is:
      s_i  = sum_v exp(l)       (ACT exp pass, accum)
      q_i  = sum_v exp(l)^2     (DVE bn_stats moments over the exp tile,
                                 q = V*(var + mean^2); ACT exp(2l) for the
                                 last batches to shorten the DVE tail)
      e0_i = exp(l[...,0]), and the gathered raw logits l[b,i,t_j].
  The gather indices/masks are pure functions of the tiny `targets` input,
  so the host precomputes them.  The device streams the 16MB logits shard,
  does all the exp/softmax-stat/chamfer-min work, and returns per-row s and
  the two per-batch min columns; the final [B,S]-level BCE scalars and means
  are finished on the host (0.02% of the FLOPs).
"""

import os
import sys

sys.path.insert(0, "/opt/trn_rl_repo")

import numpy as np

B, S, V = 64, 128, 4096
M = 8                 # NeuronCores (data-parallel over batch)
BC = B // M           # batch elements per core
R = BC * S            # rows per core
SW = S + 1            # gather width (128 targets + eos col)
EOS, PAD, EPS = 0, 4096, 1e-8
NEG = np.float32(-1e30)

_CACHE = {}


def _build_nc(reps=1, q_mode="mix", k_act=1, big_bufs=6, scr_bufs=4,
              dma_only=False, split_last=4, act_q=(1, 3)):
    # split_last: how many of the trailing batches get half-split DMAs/exps
    # act_q: extra batch indices whose q runs as an ACT exp(2l) pass
    """q_mode: 'act' = q via exp(2l) ACT pass; 'bn' = q via DVE bn_stats;
    'mix' = bn for all but the LAST k_act batches (ACT is free at the end,
    DVE is the tail bottleneck)."""
    import concourse.bacc as bacc
    import concourse.mybir as mybir
    from concourse.tile import TileContext
    from concourse.masks import make_identity

    f32 = mybir.dt.float32
    bf16 = mybir.dt.bfloat16
    A = mybir.AluOpType
    AF = mybir.ActivationFunctionType
    X = mybir.AxisListType.X

    nc = bacc.Bacc()
    lgt = nc.dram_tensor("lgt", [R, V], f32, kind="ExternalInput")
    # lgep: gathered raw logits packed [128, BC*SW]; col b*SW+S is l0
    lgep = nc.dram_tensor("lgep", [128, BC * SW], f32, kind="ExternalInput")
    mcolp = nc.dram_tensor("mcolp", [128, BC], f32, kind="ExternalInput")
    mrowp = nc.dram_tensor("mrowp", [1, BC * S], f32, kind="ExternalInput")
    # out columns: 0..BC-1 = s rows, BC..2BC-1 = min_j d2 (per i),
    # 2BC..3BC-1 = min_i d2 (per j)
    out = nc.dram_tensor("out", [128, 3 * BC], f32, kind="ExternalOutput")

    with TileContext(nc) as tc:
        with (
            tc.tile_pool(name="big", bufs=big_bufs) as bigp,
            tc.tile_pool(name="scr", bufs=scr_bufs) as scrp,
            tc.tile_pool(name="aux", bufs=2) as auxp,
            tc.tile_pool(name="sm", bufs=3) as smp,
            tc.tile_pool(name="keep", bufs=1) as keepp,
            tc.tile_pool(name="ps", bufs=2, space="PSUM") as psp,
            tc.tile_pool(name="psb", bufs=2, space="PSUM") as psbp,
        ):
            out_sb = keepp.tile([128, 3 * BC], f32, tag="outsb")
            if dma_only:
                nc.vector.memset(out_sb[:], 0.0)

            # reps>1 repeats the computation for marginal-time benchmarking
            for _rep in range(reps):
                # ---- issue every input DMA up front (13 instructions);
                # small aux first so they don't queue behind the 16MB ----
                t_lgep = auxp.tile([128, BC * SW], f32, tag="lgep")
                nc.sync.dma_start(out=t_lgep[:, :], in_=lgep[:, :])
                t_mcol = auxp.tile([128, BC], f32, tag="mcolp")
                nc.sync.dma_start(out=t_mcol[:, :], in_=mcolp[:, :])
                t_mrow = auxp.tile([1, BC * S], f32, tag="mrowp")
                nc.sync.dma_start(out=t_mrow[:, :], in_=mrowp[:, :])
                t_lgts = []
                NSPL = 2
                H = V // NSPL
                for b in range(BC):
                    t_lgt = bigp.tile([128, V], f32, tag="lgt")
                    rows = slice(b * 128, (b + 1) * 128)
                    if split_last and b >= BC - split_last:
                        # slices: earlier slices' exps overlap later DMAs
                        for h in range(NSPL):
                            cs = slice(h * H, (h + 1) * H)
                            nc.sync.dma_start(
                                out=t_lgt[:, cs], in_=lgt[rows, cs]
                            )
                    else:
                        nc.sync.dma_start(out=t_lgt[:, :], in_=lgt[rows, :])
                    t_lgts.append(t_lgt)

                if _rep == 0:
                    ones1 = keepp.tile([1, S], f32, tag="ones1")
                    nc.vector.memset(ones1[:], 1.0)
                    identp = keepp.tile([128, 128], f32, tag="identp")
                    make_identity(nc, identp[:])

                if dma_only:
                    nc.sync.dma_start(out=out[:, :], in_=out_sb[:, :])
                    continue

                # PE: broadcast all 8 mask_j rows to [128, BC*S] in PSUM via
                # two 512-wide K=1 outer products (ones^T x mrow)
                mjb_ps = []
                for h in range(2):
                    ps = psbp.tile([128, BC * S // 2], f32, tag=f"mjb{h}")
                    nc.tensor.matmul(
                        ps[:], lhsT=ones1[:],
                        rhs=t_mrow[:, h * (BC * S // 2) : (h + 1) * (BC * S // 2)],
                        start=True, stop=True,
                    )
                    mjb_ps.append(ps)

                # one exp over all gathered columns (8*129 wide)
                eg_all = auxp.tile([128, BC * SW], f32, tag="egall")
                nc.scalar.activation(eg_all[:], t_lgep[:], AF.Exp)

                for b in range(BC):
                    t_lgt = t_lgts[b]
                    mh = t_mcol[:, b : b + 1]

                    # ACT: exp pass over the [128, V] tile; accum -> s column
                    scr1 = scrp.tile([128, V], f32, tag="scr")
                    q = smp.tile([128, 1], f32, tag="q")
                    if split_last and b >= BC - split_last:
                        # per-slice s-exp (and for the very last batch also
                        # q-exp) so each slice's work starts as soon as its
                        # DMA lands
                        last = b == BC - 1
                        if last:
                            scr2 = scrp.tile([128, V], f32, tag="scr")
                            qparts = smp.tile([128, NSPL], f32, tag="qparts")
                        sparts = smp.tile([128, NSPL], f32, tag="sparts")
                        for h in range(NSPL):
                            cs = slice(h * H, (h + 1) * H)
                            nc.scalar.activation(
                                scr1[:, cs], t_lgt[:, cs], AF.Exp,
                                accum_out=sparts[:, h : h + 1],
                            )
                            if last:
                                nc.scalar.activation(
                                    scr2[:, cs], t_lgt[:, cs], AF.Exp,
                                    scale=2.0,
                                    accum_out=qparts[:, h : h + 1],
                                )
                        nc.vector.tensor_reduce(
                            out_sb[:, b : b + 1], sparts[:], axis=X, op=A.add
                        )
                        if last:
                            nc.vector.tensor_reduce(
                                q[:], qparts[:], axis=X, op=A.add
                            )
                    else:
                        nc.scalar.activation(
                            scr1[:], t_lgt[:], AF.Exp,
                            accum_out=out_sb[:, b : b + 1],
                        )
                    use_dve = (
                        (not (split_last and b == BC - 1))
                        and b not in act_q
                        and (
                            q_mode in ("bn", "ttr")
                            or (q_mode in ("mix", "mixttr") and b < BC - k_act)
                        )
                    )
                    use_ttr = q_mode in ("ttr", "mixttr")
                    if use_dve and use_ttr:
                        # q = sum(E*E) in one fused DVE pass (broadcast dummy
                        # out, the kernels/qr.py pattern)
                        dumq = smp.tile([128, 1], f32, tag="dumq")
                        nc.vector.tensor_tensor_reduce(
                            out=dumq[:].broadcast_to([128, V]), in0=scr1[:],
                            in1=scr1[:], scale=1.0, scalar=0.0,
                            op0=A.mult, op1=A.add, accum_out=q[:],
                        )
                    elif use_dve:
                        # q = sum(E^2) from bn_stats moments (tile_groupnorm
                        # pattern): 8 x 512-wide stats + aggregate
                        NSUB = V // 512
                        er = scr1[:].rearrange("p (n s) -> p n s", s=512)
                        stats = smp.tile([128, NSUB, 6], f32, tag="bnst")
                        for sg in range(NSUB):
                            nc.vector.bn_stats(
                                out=stats[:, sg, :], in_=er[:, sg, :]
                            )
                        mv = smp.tile([128, 2], f32, tag="bnmv")
                        nc.vector.bn_aggr(out=mv[:], in_=stats[:])
                        m2 = smp.tile([128, 1], f32, tag="bnm2")
                        nc.vector.tensor_mul(m2[:], mv[:, 0:1], mv[:, 0:1])
                        vpm = smp.tile([128, 1], f32, tag="bnvpm")
                        nc.vector.tensor_add(vpm[:], mv[:, 1:2], m2[:])
                        nc.vector.tensor_scalar(
                            q[:], vpm[:], float(V), None, A.mult
                        )
                    elif not (split_last and b == BC - 1):
                        scr2 = scrp.tile([128, V], f32, tag="scr")
                        nc.scalar.activation(
                            scr2[:], t_lgt[:], AF.Exp, scale=2.0,
                            accum_out=q[:],
                        )

                    eg = eg_all[:, b * SW : b * SW + S]
                    e0 = eg_all[:, b * SW + S : b * SW + S + 1]
                    # DVE: per-row softmax stats (fused two-scalar forms;
                    # signs folded so d2a ends up identical)
                    rs = smp.tile([128, 1], f32, tag="rs")
                    nc.vector.reciprocal(rs[:], out_sb[:, b : b + 1])
                    qm = smp.tile([128, 1], f32, tag="qm")
                    nc.vector.tensor_scalar(       # e0^2 - q  (= -(q-e0^2))
                        qm[:], e0, e0, q[:], A.mult, A.subtract
                    )
                    rs2m = smp.tile([128, 1], f32, tag="rs2m")
                    nc.vector.tensor_scalar(       # rs^2 * mh
                        rs2m[:], rs[:], rs[:], mh, A.mult, A.mult
                    )
                    xs = smp.tile([128, 1], f32, tag="xs")
                    nc.vector.tensor_mul(xs[:], qm[:], rs2m[:])   # = -xs_true
                    m2rsm = smp.tile([128, 1], f32, tag="m2rsm")
                    nc.vector.tensor_scalar(       # -2 * rs * mh
                        m2rsm[:], rs[:], -2.0, mh, A.mult, A.mult
                    )

                    # DVE: chamfer distance matrix and its two mins
                    mjb = mjb_ps[b // 4][:, (b % 4) * S : (b % 4 + 1) * S]
                    d2a = smp.tile([128, S], f32, tag="d2a")
                    nc.vector.tensor_scalar(       # eg*(-2 rs mh) - (-xs_true)
                        d2a[:], eg, m2rsm[:], xs[:], A.mult, A.subtract
                    )
                    # row mins need the +mask_j term elementwise
                    d2 = smp.tile([128, S], f32, tag="d2")
                    nc.vector.tensor_add(d2[:], d2a[:], mjb)
                    nc.vector.tensor_reduce(
                        out_sb[:, BC + b : BC + b + 1], d2[:], axis=X, op=A.min
                    )
                    # column mins: transpose d2a directly (doesn't wait for
                    # the mask add); in the transposed view mask_j is a
                    # per-partition constant, so add it after the min
                    # (exact: min commutes with a row-constant add)
                    pt = psp.tile([128, 128], f32, tag="pt")
                    nc.tensor.transpose(pt[:], d2a[:], identp[:])
                    ptm = smp.tile([128, 1], f32, tag="ptm")
                    nc.vector.tensor_reduce(ptm[:], pt[:], axis=X, op=A.min)
                    nc.vector.tensor_add(
                        out_sb[:, 2 * BC + b : 2 * BC + b + 1], ptm[:],
                        t_mcol[:, b : b + 1],
                    )

                nc.sync.dma_start(out=out[:, :], in_=out_sb[:, :])

    nc.compile()
    return nc


def _get_nc():
    if "nc" not in _CACHE:
        _CACHE["nc"] = _build_nc()
    return _CACHE["nc"]


def _prep(logits, targets):
    """Host-side prep: masks, counts, gathered raw logits (all derived from
    the tiny `targets` tensor + a 4MB fancy-index into logits)."""
    logits = np.ascontiguousarray(np.asarray(logits, dtype=np.float32))
    t = np.asarray(targets).astype(np.int64)
    mh = ((t != PAD) & (t != EOS)).astype(np.float32)   # eos_head
    tclip = np.minimum(t, V - 1)
    lg = np.take_along_axis(
        logits, np.broadcast_to(tclip[:, None, :], (B, S, S)), axis=2
    )
    lgm = np.where(mh[:, None, :] > 0, lg, NEG)
    lge = np.concatenate([lgm, logits[:, :, 0:1]], axis=2)       # [B,S,SW]
    return logits, lge, mh, t


def _in_maps(logits, lge, mh):
    maps = []
    for c in range(M):
        bs = slice(c * BC, (c + 1) * BC)
        # pack gathered logits as [128, BC*SW] (batch-major columns)
        lgep = np.ascontiguousarray(
            lge[bs].transpose(1, 0, 2).reshape(S, BC * SW)
        )
        maps.append(
            {
                "lgt": np.ascontiguousarray(logits[bs].reshape(R, V)),
                "lgep": lgep,
                "mcolp": np.ascontiguousarray(mh[bs].T),     # [128, BC]
                "mrowp": np.ascontiguousarray(mh[bs].reshape(1, BC * S)),
            }
        )
    return maps


def _combine(outs, logits, mh, t):
    """outs: [M][128, 3*BC] -> final [2] float32.  Finishes the reduction
    layer on the host: chamfer means from the device min columns, BCE from
    the device softmax denominators."""
    f = np.float32
    o = np.stack([np.asarray(x) for x in outs])        # [M, 128, 3*BC]
    s = o[:, :, 0:BC].transpose(0, 2, 1).reshape(B, S).astype(f)
    dmin_i = o[:, :, BC : 2 * BC].transpose(0, 2, 1).reshape(B, S)
    dmin_j = o[:, :, 2 * BC : 3 * BC].transpose(0, 2, 1).reshape(B, S)
    label = np.mean((dmin_i.sum(1) + dmin_j.sum(1)) / S)

    # BCE (host, f32, matching the reference's formulas)
    l0 = logits[:, :, 0].astype(f)
    e0 = np.exp(l0).astype(f)
    rs = (1.0 / s).astype(f)
    p0 = (e0 * rs).astype(f)
    logp = np.maximum((l0 - np.log(s).astype(f)).astype(f), f(-100.0))
    lom = np.maximum(np.log1p(-p0).astype(f), f(-100.0))
    et = (mh == 0)                                     # eos_target
    bce = np.where(et, -logp, -lom).astype(f)
    ep = (t == EOS).astype(f)
    eh = mh
    cep, ceh = ep.sum(1), eh.sum(1)
    eos = np.mean(
        0.5 * (bce * ep).sum(1) / (cep + EPS)
        + 0.5 * (bce * eh).sum(1) / (ceh + EPS)
    )
    return np.stack([label, eos]).astype(f)


def kernel(logits, targets):
    logits, lge, mh, t = _prep(logits, targets)
    maps = _in_maps(logits, lge, mh)
    nc = _get_nc()

    if os.environ.get("KMODE") == "sim":
        from concourse import bass_interp

        outs = []
        for c in range(M):
            sim = bass_interp.CoreSim(nc)
            for k, v in maps[c].items():
                sim.tensor(k)[:] = v
            sim.simulate()
            outs.append(np.array(sim.tensor("out")))
    else:
        import time

        from concourse.bass_utils import run_bass_kernel_spmd

        # the axon terminal occasionally reports a transient mesh desync;
        # a short backoff and retry recovers it
        last_err = None
        for attempt in range(3):
            try:
                res = run_bass_kernel_spmd(nc, maps, list(range(M)))
                break
            except Exception as e:  # noqa: BLE001
                last_err = e
                time.sleep(30 * (attempt + 1))
        else:
            raise last_err
        outs = [res.results[c]["out"] for c in range(M)]

    return _combine(outs, logits, mh, t)
